# revision 12
# baseline (speedup 1.0000x reference)
"""Trainium2 Bass kernel for nn_Attention2D (GroupNorm x2 -> QKV 1x1 conv ->
spatial self-attention -> out 1x1 conv -> residual, / sqrt(2)).

Contract: kernel(**inputs) takes the FULL unsharded inputs (as produced by
setup_inputs) and returns the FULL [8, 512, 64, 64] float32 output.
Internally data-parallel over batch across 8 NeuronCores.

Math notes (all hardcoded for b=8, c=512, h=w=64):
  - Both group norms are folded into a single per-channel affine A*x + B
    (per batch), derived analytically from per-channel mean/var statistics.
    The affines are then folded into the 1x1-conv weights, so projections
    consume the raw input directly.
  - softmax over rows x of scores[x, y] is computed without the max-subtract
    (scores are O(1) here), with the denominator via a ones-matmul
    (exact fp32 accumulation of the same bf16 exp values used in PV).
  - Matmuls run in bf16 (fp32 accumulate in PSUM); error << fp32 tolerance
    because the attention output is small relative to the residual branch.
"""

import os
import numpy as np

B, C, H, W = 8, 512, 64, 64
HW = H * W            # 4096
P = 128
NCH = C // P          # 4 channel chunks
G = 32                # groups
YB = 512              # attention column block
NYB = HW // YB        # 8
NXT = HW // P         # 32 row tiles
EPS = 1e-6
SCALE = float(C) ** -0.5
INV_SQRT2 = float(1.0 / np.sqrt(2.0))

_CACHE = {}


def _build_program(repeat=1, colsum_dve=True):
    import concourse.bacc as bacc
    import concourse.tile as tile
    from concourse import mybir

    f32 = mybir.dt.float32
    bf16 = mybir.dt.bfloat16
    AF = mybir.ActivationFunctionType
    OP = mybir.AluOpType

    nc = bacc.Bacc("TRN2", target_bir_lowering=False, debug=False, num_devices=8)

    q_d = nc.dram_tensor("q", [C, HW], f32, kind="ExternalInput")
    wT_d = {
        n: nc.dram_tensor(f"w{n}T", [C, C], f32, kind="ExternalInput")
        for n in ("q", "k", "v", "o")
    }
    # per-channel vectors pre-marshalled on host to [P, NCH] (col t = chans t*128..)
    vec_d = {
        n: nc.dram_tensor(n, [P, NCH], f32, kind="ExternalInput")
        for n in ("gamma", "beta", "bq", "bk", "bo")
    }
    bv_d = nc.dram_tensor("bv_row", [1, C], f32, kind="ExternalInput")
    bdiag_d = nc.dram_tensor("bdiag", [P, P], f32, kind="ExternalInput")
    out_d = nc.dram_tensor("out", [C, HW], f32, kind="ExternalOutput")

    with tile.TileContext(nc) as tc:
        with (
            tc.tile_pool(name="pers", bufs=1) as pers,
            tc.tile_pool(name="xin", bufs=2) as xinp,
            tc.tile_pool(name="stat", bufs=1) as statp,
            tc.tile_pool(name="sexp", bufs=6) as sexpp,
            tc.tile_pool(name="sout", bufs=4) as soutp,
            tc.tile_pool(name="rcpp", bufs=2) as rcpp,
            tc.tile_pool(name="accp", bufs=2) as accp,
            tc.tile_pool(name="zfp", bufs=2) as zfp,
            tc.tile_pool(name="qrp", bufs=2) as qrp,
            tc.tile_pool(name="ps_pv", bufs=4, space="PSUM") as ps_pv,
            tc.tile_pool(name="ps_s", bufs=3, space="PSUM") as ps_s,
            tc.tile_pool(name="ps_cs", bufs=1, space="PSUM") as ps_cs,
        ):
            # ---------- constants ----------
            ones_bf = pers.tile([P, P], bf16, tag="ones")
            nc.vector.memset(ones_bf[:], 1.0)
            ones_f = pers.tile([P, P], f32, tag="onesf")
            nc.vector.memset(ones_f[:], 1.0)
            # block-diagonal group-mean matrix: 16x16 blocks of 1/16 (host input)
            bdiag = pers.tile([P, P], f32, tag="bdiag")
            nc.sync.dma_start(bdiag[:], bdiag_d[:])

            eps_t = statp.tile([P, 1], f32, tag="eps")
            nc.vector.memset(eps_t[:], EPS)

            gam = statp.tile([P, NCH], f32, tag="gam")
            nc.sync.dma_start(gam[:], vec_d["gamma"][:])
            bet = statp.tile([P, NCH], f32, tag="bet")
            nc.sync.dma_start(bet[:], vec_d["beta"][:])
            bq_s = statp.tile([P, NCH], f32, tag="bq_s")
            nc.sync.dma_start(bq_s[:], vec_d["bq"][:])
            bk_s = statp.tile([P, NCH], f32, tag="bk_s")
            nc.sync.dma_start(bk_s[:], vec_d["bk"][:])
            bo_s = statp.tile([P, NCH], f32, tag="bo_s")
            nc.sync.dma_start(bo_s[:], vec_d["bo"][:])
            bv_row = statp.tile([1, C], f32, tag="bv_row")
            nc.sync.dma_start(bv_row[:], bv_d[:])

            # ---------- phase A: load x, per-channel stats, cast to bf16 ----
            x_bf = pers.tile([P, NCH, HW], bf16, tag="xbf")
            Mt = statp.tile([P, NCH], f32, tag="Mt")
            Vt = statp.tile([P, NCH], f32, tag="Vt")
            HHW = HW // 2
            for t in range(NCH):
                st = statp.tile([P, 8, 6], f32, tag=f"bnst{t}", name=f"bnst{t}")
                for h in range(2):
                    xt = xinp.tile([P, HHW], f32, tag="xin", name=f"xin{t}_{h}")
                    nc.sync.dma_start(
                        xt[:], q_d[t * P : (t + 1) * P, h * HHW : (h + 1) * HHW]
                    )
                    xr = xt.rearrange("p (n f) -> p n f", f=512)
                    for sg in range(4):
                        nc.vector.bn_stats(out=st[:, h * 4 + sg, :], in_=xr[:, sg, :])
                    nc.vector.tensor_copy(
                        x_bf[:, t, h * HHW : (h + 1) * HHW], xt[:]
                    )
                mv = statp.tile([P, 2], f32, tag=f"mv{t}", name=f"mv{t}")
                nc.vector.bn_aggr(out=mv[:], in_=st[:])
                nc.vector.tensor_copy(Mt[:, t : t + 1], mv[:, 0:1])
                nc.vector.tensor_copy(Vt[:, t : t + 1], mv[:, 1:2])

            # ---------- group-norm affine folding (all [P, NCH] f32) -------
            EX2 = statp.tile([P, NCH], f32, tag="EX2")
            nc.vector.tensor_tensor(EX2[:], Mt[:], Mt[:], OP.mult)
            nc.vector.tensor_add(EX2[:], EX2[:], Vt[:])

            R1 = statp.tile([P, 2, NCH], f32, tag="R1")
            nc.vector.tensor_copy(R1[:, 0, :], Mt[:])
            nc.vector.tensor_copy(R1[:, 1, :], EX2[:])
            G1 = ps_s.tile([P, 2 * NCH], f32, tag="s", name="G1")
            for t in range(NCH):
                nc.tensor.matmul(
                    G1[:, 2 * t : 2 * t + 2], bdiag[:], R1[:, :, t],
                    start=True, stop=True,
                )
            C1 = statp.tile([P, NCH, 2], f32, tag="C1")
            nc.vector.tensor_copy(C1[:], G1[:].rearrange("p (t s) -> p t s", s=2))
            MU1 = C1[:, :, 0]
            E1 = C1[:, :, 1]

            VAR1 = statp.tile([P, NCH], f32, tag="VAR1")
            nc.vector.tensor_tensor(VAR1[:], MU1, MU1, OP.mult)
            nc.vector.tensor_tensor(VAR1[:], E1, VAR1[:], OP.subtract)
            SD1 = statp.tile([P, NCH], f32, tag="SD1")
            nc.scalar.activation(SD1[:], VAR1[:], func=AF.Sqrt, bias=eps_t[:])
            R1D = statp.tile([P, NCH], f32, tag="R1D")
            nc.vector.reciprocal(R1D[:], SD1[:])

            A1 = statp.tile([P, NCH], f32, tag="A1")
            nc.vector.tensor_tensor(A1[:], gam[:], R1D[:], OP.mult)
            B1 = statp.tile([P, NCH], f32, tag="B1")
            nc.vector.tensor_tensor(B1[:], A1[:], MU1, OP.mult)
            nc.vector.tensor_tensor(B1[:], bet[:], B1[:], OP.subtract)

            # stats of qn = A1*x + B1
            MQ = statp.tile([P, NCH], f32, tag="MQ")
            nc.vector.tensor_tensor(MQ[:], A1[:], Mt[:], OP.mult)
            nc.vector.tensor_add(MQ[:], MQ[:], B1[:])
            E2c = statp.tile([P, NCH], f32, tag="E2c")
            nc.vector.tensor_tensor(E2c[:], A1[:], A1[:], OP.mult)
            nc.vector.tensor_tensor(E2c[:], E2c[:], Vt[:], OP.mult)
            TT = statp.tile([P, NCH], f32, tag="TT")
            nc.vector.tensor_tensor(TT[:], MQ[:], MQ[:], OP.mult)
            nc.vector.tensor_add(E2c[:], E2c[:], TT[:])

            R2 = statp.tile([P, 2, NCH], f32, tag="R2")
            nc.vector.tensor_copy(R2[:, 0, :], MQ[:])
            nc.vector.tensor_copy(R2[:, 1, :], E2c[:])
            G2 = ps_s.tile([P, 2 * NCH], f32, tag="s", name="G2")
            for t in range(NCH):
                nc.tensor.matmul(
                    G2[:, 2 * t : 2 * t + 2], bdiag[:], R2[:, :, t],
                    start=True, stop=True,
                )
            C2 = statp.tile([P, NCH, 2], f32, tag="C2")
            nc.vector.tensor_copy(C2[:], G2[:].rearrange("p (t s) -> p t s", s=2))
            MU2 = C2[:, :, 0]
            E2g = C2[:, :, 1]

            VAR2 = statp.tile([P, NCH], f32, tag="VAR2")
            nc.vector.tensor_tensor(VAR2[:], MU2, MU2, OP.mult)
            nc.vector.tensor_tensor(VAR2[:], E2g, VAR2[:], OP.subtract)
            SD2 = statp.tile([P, NCH], f32, tag="SD2")
            nc.scalar.activation(SD2[:], VAR2[:], func=AF.Sqrt, bias=eps_t[:])
            R2D = statp.tile([P, NCH], f32, tag="R2D")
            nc.vector.reciprocal(R2D[:], SD2[:])

            A2 = statp.tile([P, NCH], f32, tag="A2")
            nc.vector.tensor_tensor(A2[:], gam[:], R2D[:], OP.mult)
            nc.vector.tensor_tensor(A2[:], A2[:], A1[:], OP.mult)
            B2 = statp.tile([P, NCH], f32, tag="B2")
            nc.vector.tensor_tensor(B2[:], B1[:], MU2, OP.subtract)
            nc.vector.tensor_tensor(B2[:], B2[:], R2D[:], OP.mult)
            nc.vector.tensor_tensor(B2[:], B2[:], gam[:], OP.mult)
            nc.vector.tensor_add(B2[:], B2[:], bet[:])

            A1q = statp.tile([P, NCH], f32, tag="A1q")
            nc.vector.tensor_scalar_mul(A1q[:], A1[:], SCALE)
            B1q_bf = statp.tile([P, NCH], bf16, tag="B1q_bf")
            nc.vector.tensor_scalar_mul(B1q_bf[:], B1[:], SCALE)
            B2_bf = statp.tile([P, NCH], bf16, tag="B2_bf")
            nc.vector.tensor_copy(B2_bf[:], B2[:])

            # ---------- phase B: weights (load, cast, bias matvecs, fold) --
            wbf = {}
            for n in ("q", "k", "v", "o"):
                wb = pers.tile([P, NCH, C], bf16, tag=f"w{n}bf", name=f"w{n}bf")
                for t in range(NCH):
                    wt = xinp.tile([P, C], f32, tag="xin", name=f"wld{n}{t}")
                    nc.sync.dma_start(wt[:], wT_d[n][t * P : (t + 1) * P, :])
                    nc.vector.tensor_copy(wb[:, t, :], wt[:])
                wbf[n] = wb

            # bq2[o] = sum_c wqT[c,o]*B1q[c] + SCALE*bq[o]   (per-partition, [P, NCH])
            def bias_matvec(wb, bvec_bf, name):
                res = statp.tile([P, NCH], f32, tag=f"b2_{name}", name=f"b2_{name}")
                for ot in range(NCH):
                    pb = ps_s.tile([P, 1], f32, tag="s", name=f"pb_{name}{ot}")
                    for t in range(NCH):
                        nc.tensor.matmul(
                            pb[:], wb[:, t, ot * P : (ot + 1) * P],
                            bvec_bf[:, t : t + 1],
                            start=(t == 0), stop=(t == NCH - 1),
                        )
                    nc.vector.tensor_copy(res[:, ot : ot + 1], pb[:])
                return res

            bq2 = bias_matvec(wbf["q"], B1q_bf, "q")
            nc.vector.scalar_tensor_tensor(
                bq2[:], bq_s[:], SCALE, bq2[:], OP.mult, OP.add
            )
            bk2 = bias_matvec(wbf["k"], B2_bf, "k")
            nc.vector.tensor_add(bk2[:], bk2[:], bk_s[:])
            bo2 = statp.tile([P, NCH], f32, tag="bo2")
            nc.vector.tensor_scalar_mul(bo2[:], bo_s[:], INV_SQRT2)

            # bv2 row: bv2[co] = sum_c wvT[c,co]*B2[c] + bv[co]; broadcast to [P, C]
            pbv = ps_s.tile([1, C], f32, tag="s", name="pbv")
            for t in range(NCH):
                nc.tensor.matmul(
                    pbv[:], B2_bf[:, t : t + 1], wbf["v"][:, t, :],
                    start=(t == 0), stop=(t == NCH - 1),
                )
            bvr = statp.tile([1, C], f32, tag="bvr")
            nc.vector.tensor_copy(bvr[:], pbv[:])
            nc.vector.tensor_add(bvr[:], bvr[:], bv_row[:])
            bvr_bf = statp.tile([1, C], bf16, tag="bvr_bf")
            nc.vector.tensor_copy(bvr_bf[:], bvr[:])
            pbc = ps_s.tile([P, C], f32, tag="s", name="pbc")
            nc.tensor.matmul(pbc[:], ones_bf[0:1, :], bvr_bf[:], start=True, stop=True)
            bv2b = pers.tile([P, C], f32, tag="bv2b")
            nc.vector.tensor_copy(bv2b[:], pbc[:])

            # fold affines into weights (in place, after bias matvecs)
            for t in range(NCH):
                nc.vector.tensor_scalar_mul(
                    wbf["q"][:, t, :], wbf["q"][:, t, :], A1q[:, t : t + 1]
                )
                nc.vector.tensor_scalar_mul(
                    wbf["k"][:, t, :], wbf["k"][:, t, :], A2[:, t : t + 1]
                )
                nc.vector.tensor_scalar_mul(
                    wbf["v"][:, t, :], wbf["v"][:, t, :], A2[:, t : t + 1]
                )
                nc.vector.tensor_scalar_mul(
                    wbf["o"][:, t, :], wbf["o"][:, t, :], INV_SQRT2
                )

            # ---------- phase C: projections -------------------------------
            qp = pers.tile([P, NCH, HW], bf16, tag="qp")
            kp = pers.tile([P, NCH, HW], bf16, tag="kp")
            for dst, wname, bias in ((qp, "q", bq2), (kp, "k", bk2)):
                for ot in range(NCH):
                    for yb in range(NYB):
                        pp = ps_s.tile([P, YB], f32, tag="s", name=f"pp{wname}{ot}_{yb}")
                        for t in range(NCH):
                            nc.tensor.matmul(
                                pp[:],
                                wbf[wname][:, t, ot * P : (ot + 1) * P],
                                x_bf[:, t, yb * YB : (yb + 1) * YB],
                                start=(t == 0), stop=(t == NCH - 1),
                            )
                        nc.vector.tensor_scalar_add(
                            dst[:, ot, yb * YB : (yb + 1) * YB], pp[:],
                            bias[:, ot : ot + 1],
                        )

            vpt = pers.tile([P, NXT, C], bf16, tag="vpt")
            for xi in range(NXT):
                pv = ps_s.tile([P, C], f32, tag="s", name=f"pvt{xi}")
                for t in range(NCH):
                    nc.tensor.matmul(
                        pv[:], x_bf[:, t, xi * P : (xi + 1) * P], wbf["v"][:, t, :],
                        start=(t == 0), stop=(t == NCH - 1),
                    )
                nc.vector.tensor_add(vpt[:, xi, :], pv[:], bv2b[:])

            # ---------- phase D: attention ---------------------------------
            for rep, yb in [(r, y) for r in range(repeat) for y in range(NYB)]:
                cs = ps_cs.tile([P, YB], f32, tag="cs", name=f"cs{rep}_{yb}")
                if colsum_dve:
                    acc = accp.tile([P, YB], f32, tag="acc", name=f"acc{rep}_{yb}")
                pvacc = [
                    ps_pv.tile([P, YB], f32, tag="pv", name=f"pv{rep}_{yb}_{ct}")
                    for ct in range(NCH)
                ]
                for xi in range(NXT):
                    sp = ps_s.tile([P, YB], f32, tag="s", name=f"sp{rep}_{yb}_{xi}")
                    for t in range(NCH):
                        nc.tensor.matmul(
                            sp[:],
                            kp[:, t, xi * P : (xi + 1) * P],
                            qp[:, t, yb * YB : (yb + 1) * YB],
                            start=(t == 0), stop=(t == NCH - 1),
                        )
                    et = sexpp.tile([P, YB], bf16, tag="e", name=f"e{rep}_{yb}_{xi}")
                    nc.scalar.activation(et[:], sp[:], func=AF.Exp)
                    if colsum_dve:
                        if xi == 0:
                            nc.vector.tensor_copy(acc[:], et[:])
                        else:
                            nc.vector.tensor_add(acc[:], acc[:], et[:])
                    else:
                        nc.tensor.matmul(
                            cs[:], ones_bf[:], et[:],
                            start=(xi == 0), stop=(xi == NXT - 1),
                        )
                    for ct in range(NCH):
                        nc.tensor.matmul(
                            pvacc[ct][:], vpt[:, xi, ct * P : (ct + 1) * P], et[:],
                            start=(xi == 0), stop=(xi == NXT - 1),
                        )
                if colsum_dve:
                    nc.tensor.matmul(cs[:], ones_f[:], acc[:], start=True, stop=True)
                rc = rcpp.tile([P, YB], f32, tag="rc", name=f"rc{rep}_{yb}")
                nc.vector.reciprocal(rc[:], cs[:])
                outn = [
                    soutp.tile([P, YB], bf16, tag="on", name=f"on{rep}_{yb}_{ct}")
                    for ct in range(NCH)
                ]
                for ct in range(NCH):
                    nc.vector.tensor_tensor(outn[ct][:], pvacc[ct][:], rc[:], OP.mult)
                # wo projection + bias + residual
                for ot in range(NCH):
                    zp = ps_pv.tile([P, YB], f32, tag="pv", name=f"zp{rep}_{yb}_{ot}")
                    for ct in range(NCH):
                        nc.tensor.matmul(
                            zp[:], wbf["o"][:, ct, ot * P : (ot + 1) * P], outn[ct][:],
                            start=(ct == 0), stop=(ct == NCH - 1),
                        )
                    zt = zfp.tile([P, YB], f32, tag="z", name=f"z{rep}_{yb}_{ot}")
                    nc.vector.tensor_scalar_add(zt[:], zp[:], bo2[:, ot : ot + 1])
                    qt = qrp.tile([P, YB], f32, tag="qr", name=f"qr{rep}_{yb}_{ot}")
                    nc.sync.dma_start(
                        qt[:], q_d[ot * P : (ot + 1) * P, yb * YB : (yb + 1) * YB]
                    )
                    nc.vector.scalar_tensor_tensor(
                        qt[:], qt[:], INV_SQRT2, zt[:], OP.mult, OP.add
                    )
                    nc.sync.dma_start(
                        out_d[ot * P : (ot + 1) * P, yb * YB : (yb + 1) * YB], qt[:]
                    )

    nc.compile()
    return nc


def _get_program(repeat=1, colsum_dve=True):
    key = f"nc{repeat}_{colsum_dve}"
    if key not in _CACHE:
        _CACHE[key] = _build_program(repeat, colsum_dve)
    return _CACHE[key]


def _marshal(q, gamma_q, beta_q, wq, bq, wk, bk, wv, bv, wo, bo):
    def pt(v):  # [C] -> [P, NCH] with col t = channels t*128..(t+1)*128
        return np.ascontiguousarray(
            np.asarray(v, np.float32).reshape(NCH, P).T
        )

    shared = {
        "wqT": np.ascontiguousarray(np.asarray(wq, np.float32).T),
        "wkT": np.ascontiguousarray(np.asarray(wk, np.float32).T),
        "wvT": np.ascontiguousarray(np.asarray(wv, np.float32).T),
        "woT": np.ascontiguousarray(np.asarray(wo, np.float32).T),
        "gamma": pt(gamma_q),
        "beta": pt(beta_q),
        "bq": pt(bq),
        "bk": pt(bk),
        "bo": pt(bo),
        "bv_row": np.ascontiguousarray(np.asarray(bv, np.float32).reshape(1, C)),
        "bdiag": np.kron(np.eye(8), np.full((16, 16), 1.0 / 16)).astype(np.float32),
    }
    q = np.asarray(q, np.float32)
    in_maps = []
    for c in range(B):
        m = dict(shared)
        m["q"] = np.ascontiguousarray(q[c].reshape(C, HW))
        in_maps.append(m)
    return in_maps


def run(inputs, trace=False, repeat=1):
    from concourse.bass_utils import run_bass_kernel_spmd

    nc = _get_program(repeat)
    in_maps = _marshal(**inputs)
    res = run_bass_kernel_spmd(nc, in_maps, list(range(B)), trace=trace)
    out = np.stack([res.results[c]["out"] for c in range(B)], axis=0)
    return out.reshape(B, C, H, W), res


def kernel(**inputs) -> np.ndarray:
    out, _ = run(inputs, trace=False)
    return out
